# revision 30
# baseline (speedup 1.0000x reference)
"""Chamfer distance kernel for Trainium2 (Bass/Tile), SPMD over 8 NeuronCores.

Problem: source [8, 4096, 3], target [8, 4096, 3] float32.
  distance[b, n, m] = sum_c (source[b,n,c] - target[b,m,c])^2
  loss_src = mean_n min_m distance ; loss_dst = mean_m min_n distance
  returns (loss_src, loss_dst)

Sharding: batch b -> core b (data parallel, no cross-core comms until the
final host-side mean).

Per-core algorithm:
  d[n, m] = ||s_n||^2 - 2 s_n.t_m + ||t_m||^2 expressed as a K=16 bf16
  matmul U[:, n] . V[:, m] where every fp32 input is split into a
  bf16 hi + bf16 lo pair (products are exact in the fp32 PSUM accumulator,
  so the only error is the dropped >=2nd-order residual, ~1e-7, plus the
  final bf16 cast of d, ~0.2% relative on each distance, which averages
  out over the 4096-term means).

  For each 128-row tile of n: 8 bf16 matmuls [16,128]x[16,512] produce the
  d tile [128, 4096] in PSUM (two [128,2048] half-tiles, double-buffered);
  ScalarE copies each half to SBUF casting to bf16; VectorE then:
    - col-min: tensor_tensor min into a persistent [128, 4096] accumulator
    - row-min: one tensor_tensor min fold (4096 -> 2048) into a slot of a
      [128, 8, 2048] group buffer; every 8 tiles one strided tensor_reduce
      yields those 8 tiles' row minima (amortizes the 1x-rate reduce).
  Tail: partition-axis min of the accumulator via 32x32 stream transpose,
  strided min-reduce, SBUF->SBUF DMA regroup, and two free-dim folds.
  Host: means over the returned row/col minima (final mean only).
"""

import os
import sys

import numpy as np

_TRN_REPO = "/opt/trn_rl_repo"
if _TRN_REPO not in sys.path and os.path.isdir(_TRN_REPO):
    sys.path.insert(0, _TRN_REPO)

from contextlib import ExitStack

import ml_dtypes

import concourse.bacc as bacc
import concourse.bass as bass
import concourse.mybir as mybir
import concourse.tile as tile

F32 = mybir.dt.float32
BF16 = mybir.dt.bfloat16
MIN = mybir.AluOpType.min
BF16NP = ml_dtypes.bfloat16
MIN_INIT = 1e30
K_AUG = 16  # rows of the split-precision augmented factors

# full problem shape (hardcoded: harness runs kernel.py standalone)
B, N, M, C = 8, 4096, 4096, 3
N_CORES = 8
GROUP = 8  # n-tiles per grouped row-min reduce


def build_chamfer_nc(n: int = N, m: int = M, m_chunk: int = 2048, group: int = GROUP, pe_tail: bool = True, init_copy: bool = False):
    """Build the per-core Bass program. n: source points, m: target points."""
    assert n % 128 == 0 and m % m_chunk == 0 and m % 64 == 0 and m_chunk % 512 == 0
    n_tiles = n // 128
    n_chunks = m // m_chunk
    group = min(group, n_tiles)
    assert n_tiles % group == 0

    nc = bacc.Bacc("TRN2", target_bir_lowering=False, debug=False)
    u_d = nc.dram_tensor("u_in", [K_AUG, n], BF16, kind="ExternalInput").ap()
    v_d = nc.dram_tensor("v_in", [K_AUG, m], BF16, kind="ExternalInput").ap()
    eye_d = nc.dram_tensor("eye_in", [128, 128], BF16, kind="ExternalInput").ap()
    row_d = nc.dram_tensor("row_out", [128, n_tiles], F32, kind="ExternalOutput").ap()
    col_d = nc.dram_tensor("col_out", [128, m // 128], F32, kind="ExternalOutput").ap()

    with tile.TileContext(nc) as tc, ExitStack() as ctx:
        const_pool = ctx.enter_context(tc.tile_pool(name="const", bufs=1))
        psum_pool = ctx.enter_context(tc.tile_pool(name="psum", bufs=2, space="PSUM"))
        d_pool = ctx.enter_context(tc.tile_pool(name="dtiles", bufs=3))
        g_pool = ctx.enter_context(tc.tile_pool(name="gbuf", bufs=2))
        f_pool = ctx.enter_context(tc.tile_pool(name="folds", bufs=1))
        c_pool = ctx.enter_context(tc.tile_pool(name="ctree", bufs=1))
        scratch_pool = ctx.enter_context(tc.tile_pool(name="scratch", bufs=2))

        # initial loads spread over independent DMA queues so they overlap
        u_t = const_pool.tile([K_AUG, n], BF16, tag="u")
        nc.sync.dma_start(u_t[:], u_d[:])
        v_t = const_pool.tile([K_AUG, m], BF16, tag="v")
        for q in range(4):
            lo, hi = q * (m // 4), (q + 1) * (m // 4)
            eng = nc.scalar if q % 2 == 0 else nc.gpsimd
            eng.dma_start(v_t[:, lo:hi], v_d[:, lo:hi])

        eye_t = const_pool.tile([128, 128], BF16, tag="eye")
        nc.gpsimd.dma_start(eye_t[:], eye_d[:])

        acc = const_pool.tile([128, m], BF16, tag="acc")
        if not init_copy:
            nc.vector.memset(acc[:], MIN_INIT)
        rowmins = const_pool.tile([128, n_tiles], F32, tag="rowmins")

        fold_w = max(m // 32, 64)  # row-min folded down to this width per tile
        quad = 4 if n_tiles % 4 == 0 and group % 4 == 0 else 1
        assert group % quad == 0
        gbuf = None
        for q in range(n_tiles // quad):
            if (q * quad) % group == 0:
                gbuf = g_pool.tile([128, group, fold_w], BF16, tag="gbuf")
            dd = d_pool.tile([128, quad, m], BF16, tag="d_sb")
            for t in range(quad):
                i = q * quad + t
                for h in range(n_chunks):
                    ps = psum_pool.tile([128, m_chunk], F32, tag="ps")
                    for j in range(m_chunk // 512):
                        mm = h * m_chunk + j * 512
                        nc.tensor.matmul(
                            ps[:, j * 512 : (j + 1) * 512],
                            u_t[:, i * 128 : (i + 1) * 128],
                            v_t[:, mm : mm + 512],
                            start=True,
                            stop=True,
                        )
                    nc.scalar.copy(dd[:, t, h * m_chunk : (h + 1) * m_chunk], ps[:])

                # col-min: one 3D pair-min over the whole quad, then join
                # (3 ops per quad instead of 4)
                if quad == 4:
                    if t == 3:
                        cp = c_pool.tile([128, 2, m], BF16, tag="cp")
                        nc.vector.tensor_tensor(
                            cp[:], dd[:, 0:2, :], dd[:, 2:4, :], MIN
                        )
                        if i == n_tiles - 1:
                            # split so the tail transposes start on the low half
                            nc.vector.tensor_tensor(
                                cp[:, 0, : m // 2], cp[:, 0, : m // 2],
                                cp[:, 1, : m // 2], MIN,
                            )
                            nc.vector.tensor_tensor(
                                acc[:, : m // 2], acc[:, : m // 2],
                                cp[:, 0, : m // 2], MIN,
                            )
                            nc.vector.tensor_tensor(
                                cp[:, 0, m // 2 :], cp[:, 0, m // 2 :],
                                cp[:, 1, m // 2 :], MIN,
                            )
                            nc.vector.tensor_tensor(
                                acc[:, m // 2 :], acc[:, m // 2 :],
                                cp[:, 0, m // 2 :], MIN,
                            )
                        else:
                            nc.vector.tensor_tensor(
                                cp[:, 0, :], cp[:, 0, :], cp[:, 1, :], MIN
                            )
                            nc.vector.tensor_tensor(acc[:], acc[:], cp[:, 0, :], MIN)
                else:
                    nc.vector.tensor_tensor(acc[:], acc[:], dd[:, t, :], MIN)

            # row-min fold chain m -> fold_w for the whole quad at once
            # (3D APs amortize the per-op fixed cost over `quad` tiles)
            prev = dd[:, :, :]
            w = m
            lvl = 0
            while w > 2 * fold_w:
                w //= 2
                lvl += 1
                f = f_pool.tile([128, quad, w], BF16, tag=f"f{lvl}")
                nc.vector.tensor_tensor(f[:], prev[:, :, :w], prev[:, :, w:], MIN)
                prev = f
            s0 = (q * quad) % group
            nc.vector.tensor_tensor(
                gbuf[:, s0 : s0 + quad, :], prev[:, :, :fold_w], prev[:, :, fold_w:], MIN
            )

            if (q * quad + quad) % group == 0:
                # one strided reduce finishes row minima for `group` tiles
                g0 = (q * quad + quad) - group
                nc.vector.tensor_reduce(
                    rowmins[:, g0 : g0 + group],
                    gbuf[:],
                    axis=mybir.AxisListType.X,
                    op=MIN,
                )

        # ---- tail: reduce acc over the 128 partitions -> col minima ----
        # PE full-128x128 transposes (bf16 -> PSUM) + one strided 1x reduce:
        # T_k[p, j] = acc[j, 128k + p]  =>  colmins[p, k] = min_j T_k[p, j]
        n_blk = m // 128
        assert pe_tail
        colmins = scratch_pool.tile([128, n_blk], F32, tag="colmins")
        per = 16  # transposed blocks per PSUM tile ([128, 16*128] bf16 = 2 banks)
        for c in range(n_blk // per):
            psT = psum_pool.tile([128, per * 128], BF16, tag="ps")
            for k in range(per):
                blk = c * per + k
                nc.tensor.transpose(
                    psT[:, k * 128 : (k + 1) * 128],
                    acc[:, blk * 128 : (blk + 1) * 128],
                    eye_t[:],
                )
            nc.vector.tensor_reduce(
                colmins[:, c * per : (c + 1) * per],
                psT[:].rearrange("p (k j) -> p k j", j=128),
                axis=mybir.AxisListType.X,
                op=MIN,
            )

        nc.sync.dma_start(row_d[:], rowmins[:])
        nc.sync.dma_start(col_d[:], colmins[:])

    nc.compile()
    return nc


def _split_bf16(x):
    """x (f32/f64) -> (hi, lo) bf16 pair with hi + lo ~= x."""
    x = np.asarray(x, np.float32)
    hi = x.astype(BF16NP)
    lo = (x - hi.astype(np.float32)).astype(BF16NP)
    return hi, lo


def make_uv(source: np.ndarray, target: np.ndarray):
    """Host prep: U [B, 16, N], V [B, 16, M] bf16 split-precision factors.

    d[n,m] = sum_k U[k,n] V[k,m]:
      k 0-2 : sh_c       * (-2 th_c)
      k 3-5 : sh_c       * (-2 tl_c)
      k 6-8 : sl_c       * (-2 th_c)
      k 9-11: sl_c       * (-2 tl_c)
      k 12  : ah          * 1         (a = ||s||^2 = ah + al)
      k 13  : al          * 1
      k 14  : 1           * bh        (b = ||t||^2 = bh + bl)
      k 15  : 1           * bl
    """
    s = np.asarray(source, np.float32)
    t = np.asarray(target, np.float32)
    b, n, _ = s.shape
    m = t.shape[1]
    sh, sl = _split_bf16(s)  # [B, N, 3]
    th, tl = _split_bf16(t)
    a = (s.astype(np.float64) ** 2).sum(-1)
    bb = (t.astype(np.float64) ** 2).sum(-1)
    ah, al = _split_bf16(a)
    bh, bl = _split_bf16(bb)

    u = np.zeros((b, K_AUG, n), BF16NP)
    v = np.zeros((b, K_AUG, m), BF16NP)
    u[:, 0:3] = sh.transpose(0, 2, 1)
    u[:, 3:6] = sh.transpose(0, 2, 1)
    u[:, 6:9] = sl.transpose(0, 2, 1)
    u[:, 9:12] = sl.transpose(0, 2, 1)
    u[:, 12] = ah
    u[:, 13] = al
    u[:, 14] = 1.0
    u[:, 15] = 1.0
    # -2 * bf16 value is exact in bf16
    v[:, 0:3] = (-2.0 * th.astype(np.float32)).astype(BF16NP).transpose(0, 2, 1)
    v[:, 3:6] = (-2.0 * tl.astype(np.float32)).astype(BF16NP).transpose(0, 2, 1)
    v[:, 6:9] = v[:, 0:3]
    v[:, 9:12] = v[:, 3:6]
    v[:, 12] = 1.0
    v[:, 13] = 1.0
    v[:, 14] = bh
    v[:, 15] = bl
    return u, v


_NC_CACHE = {}


def _get_nc():
    key = (N, M)
    if key not in _NC_CACHE:
        _NC_CACHE[key] = build_chamfer_nc(N, M)
    return _NC_CACHE[key]


def run_device(u: np.ndarray, v: np.ndarray, trace: bool = False):
    """u,v: [B, 16, N/M] bf16. Returns (rowmins [B, N], colmins [B, M], results)."""
    from concourse.bass_utils import run_bass_kernel_spmd

    nc = _get_nc()
    eye = np.eye(128, dtype=BF16NP)
    in_maps = [{"u_in": u[c], "v_in": v[c], "eye_in": eye} for c in range(N_CORES)]
    res = run_bass_kernel_spmd(nc, in_maps, list(range(N_CORES)), trace=trace)
    rowmins = np.stack(
        [res.results[c]["row_out"].T.reshape(-1) for c in range(N_CORES)]
    )  # row_out[p, i] = rowmin(n = 128 i + p) -> .T flat gives n = 128 i + p
    colmins = np.stack(
        [res.results[c]["col_out"].T.reshape(-1) for c in range(N_CORES)]
    )  # col_out[p, k] = colmin(m = 128 k + p) -> .T flat gives m = 128 k + p
    return rowmins, colmins, res


def kernel(source: np.ndarray, target: np.ndarray):
    u, v = make_uv(source, target)
    rowmins, colmins, _ = run_device(u, v)
    loss_src = np.float32(rowmins.mean(dtype=np.float64))
    loss_dst = np.float32(colmins.mean(dtype=np.float64))
    return (loss_src, loss_dst)


# revision 31
# speedup vs baseline: 1.0419x; 1.0419x over previous
"""Chamfer distance kernel for Trainium2 (Bass/Tile), SPMD over 8 NeuronCores.

Problem: source [8, 4096, 3], target [8, 4096, 3] float32.
  distance[b, n, m] = sum_c (source[b,n,c] - target[b,m,c])^2
  loss_src = mean_n min_m distance ; loss_dst = mean_m min_n distance
  returns (loss_src, loss_dst)

Sharding: batch b -> core b (data parallel, no cross-core comms until the
final host-side mean).

Per-core algorithm:
  d[n, m] = ||s_n||^2 - 2 s_n.t_m + ||t_m||^2 expressed as a K=16 bf16
  matmul U[:, n] . V[:, m] where every fp32 input is split into a
  bf16 hi + bf16 lo pair (products are exact in the fp32 PSUM accumulator,
  so the only error is the dropped >=2nd-order residual, ~1e-7, plus the
  final bf16 cast of d, ~0.2% relative on each distance, which averages
  out over the 4096-term means).

  For each 128-row tile of n: 8 bf16 matmuls [16,128]x[16,512] produce the
  d tile [128, 4096] in PSUM (two [128,2048] half-tiles, double-buffered);
  ScalarE copies each half to SBUF casting to bf16; VectorE then:
    - col-min: tensor_tensor min into a persistent [128, 4096] accumulator
    - row-min: one tensor_tensor min fold (4096 -> 2048) into a slot of a
      [128, 8, 2048] group buffer; every 8 tiles one strided tensor_reduce
      yields those 8 tiles' row minima (amortizes the 1x-rate reduce).
  Tail: partition-axis min of the accumulator via 32x32 stream transpose,
  strided min-reduce, SBUF->SBUF DMA regroup, and two free-dim folds.
  Host: means over the returned row/col minima (final mean only).
"""

import os
import sys

import numpy as np

_TRN_REPO = "/opt/trn_rl_repo"
if _TRN_REPO not in sys.path and os.path.isdir(_TRN_REPO):
    sys.path.insert(0, _TRN_REPO)

from contextlib import ExitStack

import ml_dtypes

import concourse.bacc as bacc
import concourse.bass as bass
import concourse.mybir as mybir
import concourse.tile as tile

F32 = mybir.dt.float32
BF16 = mybir.dt.bfloat16
MIN = mybir.AluOpType.min
BF16NP = ml_dtypes.bfloat16
MIN_INIT = 1e30
K_AUG = 16  # rows of the split-precision augmented factors

# full problem shape (hardcoded: harness runs kernel.py standalone)
B, N, M, C = 8, 4096, 4096, 3
N_CORES = 8
GROUP = 8  # n-tiles per grouped row-min reduce


def build_chamfer_nc(n: int = N, m: int = M, m_chunk: int = 2048, group: int = GROUP, pe_tail: bool = True, init_copy: bool = False):
    """Build the per-core Bass program. n: source points, m: target points."""
    assert n % 128 == 0 and m % m_chunk == 0 and m % 64 == 0 and m_chunk % 512 == 0
    n_tiles = n // 128
    n_chunks = m // m_chunk
    group = min(group, n_tiles)
    assert n_tiles % group == 0

    nc = bacc.Bacc("TRN2", target_bir_lowering=False, debug=False)
    u_d = nc.dram_tensor("u_in", [K_AUG, n], BF16, kind="ExternalInput").ap()
    v_d = nc.dram_tensor("v_in", [K_AUG, m], BF16, kind="ExternalInput").ap()
    eye_d = nc.dram_tensor("eye_in", [128, 128], BF16, kind="ExternalInput").ap()
    row_d = nc.dram_tensor("row_out", [128, n_tiles], F32, kind="ExternalOutput").ap()
    col_d = nc.dram_tensor("col_out", [128, m // 128], F32, kind="ExternalOutput").ap()

    with tile.TileContext(nc) as tc, ExitStack() as ctx:
        const_pool = ctx.enter_context(tc.tile_pool(name="const", bufs=1))
        psum_pool = ctx.enter_context(tc.tile_pool(name="psum", bufs=2, space="PSUM"))
        d_pool = ctx.enter_context(tc.tile_pool(name="dtiles", bufs=3))
        g_pool = ctx.enter_context(tc.tile_pool(name="gbuf", bufs=2))
        f_pool = ctx.enter_context(tc.tile_pool(name="folds", bufs=1))
        c_pool = ctx.enter_context(tc.tile_pool(name="ctree", bufs=1))
        scratch_pool = ctx.enter_context(tc.tile_pool(name="scratch", bufs=2))

        # initial loads spread over independent DMA queues so they overlap
        u_t = const_pool.tile([K_AUG, n], BF16, tag="u")
        nc.sync.dma_start(u_t[:], u_d[:])
        v_t = const_pool.tile([K_AUG, m], BF16, tag="v")
        for q in range(4):
            lo, hi = q * (m // 4), (q + 1) * (m // 4)
            eng = nc.scalar if q % 2 == 0 else nc.gpsimd
            eng.dma_start(v_t[:, lo:hi], v_d[:, lo:hi])

        eye_t = const_pool.tile([128, 128], BF16, tag="eye")
        nc.gpsimd.dma_start(eye_t[:], eye_d[:])

        acc = const_pool.tile([128, m], BF16, tag="acc")
        if not init_copy:
            nc.vector.memset(acc[:], MIN_INIT)
        rowmins = const_pool.tile([128, n_tiles], F32, tag="rowmins")

        fold_w = max(m // 32, 64)  # row-min folded down to this width per tile
        quad = 4 if n_tiles % 4 == 0 and group % 4 == 0 else 1
        assert group % quad == 0
        gbuf = None
        for q in range(n_tiles // quad):
            if (q * quad) % group == 0:
                gbuf = g_pool.tile([128, group, fold_w], BF16, tag="gbuf")
            dd = d_pool.tile([128, quad, m], BF16, tag="d_sb")
            for t in range(quad):
                i = q * quad + t
                for h in range(n_chunks):
                    ps = psum_pool.tile([128, m_chunk], F32, tag="ps")
                    for j in range(m_chunk // 512):
                        mm = h * m_chunk + j * 512
                        nc.tensor.matmul(
                            ps[:, j * 512 : (j + 1) * 512],
                            u_t[:, i * 128 : (i + 1) * 128],
                            v_t[:, mm : mm + 512],
                            start=True,
                            stop=True,
                        )
                    nc.scalar.copy(dd[:, t, h * m_chunk : (h + 1) * m_chunk], ps[:])

                # col-min accumulate per tile (keeps DVE fed while the quad fills)
                if i in (0, n_tiles - 1):
                    # split first (earlier DVE start) and last (tail overlap)
                    nc.vector.tensor_tensor(
                        acc[:, : m // 2], acc[:, : m // 2], dd[:, t, : m // 2], MIN
                    )
                    nc.vector.tensor_tensor(
                        acc[:, m // 2 :], acc[:, m // 2 :], dd[:, t, m // 2 :], MIN
                    )
                else:
                    nc.vector.tensor_tensor(acc[:], acc[:], dd[:, t, :], MIN)

            # row-min fold chain m -> fold_w for the whole quad at once
            # (3D APs amortize the per-op fixed cost over `quad` tiles)
            prev = dd[:, :, :]
            w = m
            lvl = 0
            while w > 2 * fold_w:
                w //= 2
                lvl += 1
                f = f_pool.tile([128, quad, w], BF16, tag=f"f{lvl}")
                nc.vector.tensor_tensor(f[:], prev[:, :, :w], prev[:, :, w:], MIN)
                prev = f
            s0 = (q * quad) % group
            nc.vector.tensor_tensor(
                gbuf[:, s0 : s0 + quad, :], prev[:, :, :fold_w], prev[:, :, fold_w:], MIN
            )

            if (q * quad + quad) % group == 0:
                # one strided reduce finishes row minima for `group` tiles
                g0 = (q * quad + quad) - group
                nc.vector.tensor_reduce(
                    rowmins[:, g0 : g0 + group],
                    gbuf[:],
                    axis=mybir.AxisListType.X,
                    op=MIN,
                )

        # ---- tail: reduce acc over the 128 partitions -> col minima ----
        # PE full-128x128 transposes (bf16 -> PSUM) + one strided 1x reduce:
        # T_k[p, j] = acc[j, 128k + p]  =>  colmins[p, k] = min_j T_k[p, j]
        n_blk = m // 128
        assert pe_tail
        colmins = scratch_pool.tile([128, n_blk], F32, tag="colmins")
        per = 16  # transposed blocks per PSUM tile ([128, 16*128] bf16 = 2 banks)
        for c in range(n_blk // per):
            psT = psum_pool.tile([128, per * 128], BF16, tag="ps")
            for k in range(per):
                blk = c * per + k
                nc.tensor.transpose(
                    psT[:, k * 128 : (k + 1) * 128],
                    acc[:, blk * 128 : (blk + 1) * 128],
                    eye_t[:],
                )
            nc.vector.tensor_reduce(
                colmins[:, c * per : (c + 1) * per],
                psT[:].rearrange("p (k j) -> p k j", j=128),
                axis=mybir.AxisListType.X,
                op=MIN,
            )

        nc.sync.dma_start(row_d[:], rowmins[:])
        nc.sync.dma_start(col_d[:], colmins[:])

    nc.compile()
    return nc


def _split_bf16(x):
    """x (f32/f64) -> (hi, lo) bf16 pair with hi + lo ~= x."""
    x = np.asarray(x, np.float32)
    hi = x.astype(BF16NP)
    lo = (x - hi.astype(np.float32)).astype(BF16NP)
    return hi, lo


def make_uv(source: np.ndarray, target: np.ndarray):
    """Host prep: U [B, 16, N], V [B, 16, M] bf16 split-precision factors.

    d[n,m] = sum_k U[k,n] V[k,m]:
      k 0-2 : sh_c       * (-2 th_c)
      k 3-5 : sh_c       * (-2 tl_c)
      k 6-8 : sl_c       * (-2 th_c)
      k 9-11: sl_c       * (-2 tl_c)
      k 12  : ah          * 1         (a = ||s||^2 = ah + al)
      k 13  : al          * 1
      k 14  : 1           * bh        (b = ||t||^2 = bh + bl)
      k 15  : 1           * bl
    """
    s = np.asarray(source, np.float32)
    t = np.asarray(target, np.float32)
    b, n, _ = s.shape
    m = t.shape[1]
    sh, sl = _split_bf16(s)  # [B, N, 3]
    th, tl = _split_bf16(t)
    a = (s.astype(np.float64) ** 2).sum(-1)
    bb = (t.astype(np.float64) ** 2).sum(-1)
    ah, al = _split_bf16(a)
    bh, bl = _split_bf16(bb)

    u = np.zeros((b, K_AUG, n), BF16NP)
    v = np.zeros((b, K_AUG, m), BF16NP)
    u[:, 0:3] = sh.transpose(0, 2, 1)
    u[:, 3:6] = sh.transpose(0, 2, 1)
    u[:, 6:9] = sl.transpose(0, 2, 1)
    u[:, 9:12] = sl.transpose(0, 2, 1)
    u[:, 12] = ah
    u[:, 13] = al
    u[:, 14] = 1.0
    u[:, 15] = 1.0
    # -2 * bf16 value is exact in bf16
    v[:, 0:3] = (-2.0 * th.astype(np.float32)).astype(BF16NP).transpose(0, 2, 1)
    v[:, 3:6] = (-2.0 * tl.astype(np.float32)).astype(BF16NP).transpose(0, 2, 1)
    v[:, 6:9] = v[:, 0:3]
    v[:, 9:12] = v[:, 3:6]
    v[:, 12] = 1.0
    v[:, 13] = 1.0
    v[:, 14] = bh
    v[:, 15] = bl
    return u, v


_NC_CACHE = {}


def _get_nc():
    key = (N, M)
    if key not in _NC_CACHE:
        _NC_CACHE[key] = build_chamfer_nc(N, M)
    return _NC_CACHE[key]


def run_device(u: np.ndarray, v: np.ndarray, trace: bool = False):
    """u,v: [B, 16, N/M] bf16. Returns (rowmins [B, N], colmins [B, M], results)."""
    from concourse.bass_utils import run_bass_kernel_spmd

    nc = _get_nc()
    eye = np.eye(128, dtype=BF16NP)
    in_maps = [{"u_in": u[c], "v_in": v[c], "eye_in": eye} for c in range(N_CORES)]
    res = run_bass_kernel_spmd(nc, in_maps, list(range(N_CORES)), trace=trace)
    rowmins = np.stack(
        [res.results[c]["row_out"].T.reshape(-1) for c in range(N_CORES)]
    )  # row_out[p, i] = rowmin(n = 128 i + p) -> .T flat gives n = 128 i + p
    colmins = np.stack(
        [res.results[c]["col_out"].T.reshape(-1) for c in range(N_CORES)]
    )  # col_out[p, k] = colmin(m = 128 k + p) -> .T flat gives m = 128 k + p
    return rowmins, colmins, res


def kernel(source: np.ndarray, target: np.ndarray):
    u, v = make_uv(source, target)
    rowmins, colmins, _ = run_device(u, v)
    loss_src = np.float32(rowmins.mean(dtype=np.float64))
    loss_dst = np.float32(colmins.mean(dtype=np.float64))
    return (loss_src, loss_dst)
